# revision 1
# baseline (speedup 1.0000x reference)
"""Trainium2 Bass kernel for nn_Attention_33354716021131.

Dense GQA attention block (B=2, S=2048, D=4096, 32 q-heads / 8 kv-heads,
head_dim 128, RoPE, causal softmax) tensor-parallel across 8 NeuronCores.

Sharding (per core c):
  - heads: q-heads 4c..4c+3 (one kv-head group c) -> wq/wk/wv column shards
  - x transposed cooperatively: core c transposes x[:, 512c:512c+512] on the
    PE, AllGather -> full x^T on every core
  - attention entirely local to the core (its 4 q-heads x 2 batches)
  - attention outputs (head-major, transposed) AllGather -> full O^T, then
    wo column shard: core c computes y[:, 512c:512c+512]; host concatenates.

Everything stays in "transposed" [feature, token] layout between the input
transpose and the final wo projection, which makes every matmul a clean
[K=128 partition] x [N=512 free] fp32r instruction:
  - QKV:   qT/kT/vT tile = w_tile.T @ xT_tile            (accumulate over k)
  - RoPE:  pair-swap via a constant permutation matmul, cos/sin via DVE
  - S^T:   sT[k,q] = kT_tile.T @ qT_tile   (keys on partitions)
  - P^T:   exp on ScalarE (scale fused), causal tri-mask on diagonal tiles
  - PV:    oT[d,q] += v_nat_tile.T @ pT_tile; denominators via ones-matmul
  - WO:    y[tok, cols] = oT_tile.T @ wo_tile
All matmul operands are float32r (fp32 bits, PE reduced-precision mode,
4x faster than fp32; measured ~1.5e-4 rel err at K=4096).
"""
import math
import os

import numpy as np

N_CORES = 8
B = 2
S = 2048
DM = 4096
N_HEADS = 32
HD = 128
NQH = N_HEADS // N_CORES          # 4 q heads per core
HDQ = NQH * HD                    # 512
T = B * S                         # 4096 tokens
KC = DM // 128                    # 32 k-chunks
TB = 512                          # token block for projections
NTB = S // TB                     # 4 per batch
QB = 512                          # query block for attention
NQB = S // QB                     # 4
NKT = S // 128                    # 16 key tiles per batch
SCALE = 1.0 / math.sqrt(HD)
ROPE_THETA = 10000.0

_CACHE = {}


def _consts():
    i = np.arange(HD // 2)
    inv = 1.0 / (ROPE_THETA ** (2 * i / HD))
    t = np.arange(S)
    ang = np.outer(inv, t)  # [64, S]
    cosT = np.repeat(np.cos(ang), 2, axis=0).astype(np.float32)  # [128, S]
    sinT = np.repeat(np.sin(ang), 2, axis=0).astype(np.float32)
    perm = np.zeros((128, 128), np.float32)
    for j in range(64):
        perm[2 * j, 2 * j + 1] = 1.0
        perm[2 * j + 1, 2 * j] = -1.0
    tri = (np.arange(128)[:, None] <= np.arange(128)[None, :]).astype(np.float32)
    ident = np.eye(128, dtype=np.float32)
    ones = np.ones((128, 1), np.float32)
    return cosT, sinT, perm, tri, ident, ones


STAGES = ["p0", "xag", "qkv0", "attn0", "oag0", "qkv1", "attn1", "oag1", "wo"]


def _build(sim=False, repeat=1, phase_reps=(1, 1, 1, 1), until="wo"):
    import concourse.mybir as mybir
    import concourse.tile as tile
    from concourse import bacc

    F32 = mybir.dt.float32
    F32R = mybir.dt.float32r

    nc = bacc.Bacc("TRN2", target_bir_lowering=False, debug=False,
                   num_devices=N_CORES)

    xs = nc.dram_tensor("xs", [T, HDQ], F32, kind="ExternalInput")
    wq = nc.dram_tensor("wq", [DM, HDQ], F32, kind="ExternalInput")
    wk = nc.dram_tensor("wk", [DM, HD], F32, kind="ExternalInput")
    wv = nc.dram_tensor("wv", [DM, HD], F32, kind="ExternalInput")
    wo = nc.dram_tensor("wo", [DM, HDQ], F32, kind="ExternalInput")
    cosc = nc.dram_tensor("cosc", [128, S], F32, kind="ExternalInput")
    sinc = nc.dram_tensor("sinc", [128, S], F32, kind="ExternalInput")
    permc = nc.dram_tensor("permc", [128, 128], F32, kind="ExternalInput")
    tric = nc.dram_tensor("tric", [128, 128], F32, kind="ExternalInput")
    identc = nc.dram_tensor("identc", [128, 128], F32, kind="ExternalInput")
    onesc = nc.dram_tensor("onesc", [128, 1], F32, kind="ExternalInput")

    y = nc.dram_tensor("y", [T, HDQ], F32, kind="ExternalOutput")

    rg = [list(range(N_CORES))]

    with tile.TileContext(nc) as tc:
        with (
            tc.tile_pool(name="dram", bufs=1, space="DRAM") as dram,
            tc.tile_pool(name="const", bufs=1) as cp,
        ):
            cos_sb = cp.tile([128, S], F32, tag="cos")
            sin_sb = cp.tile([128, S], F32, tag="sin")
            perm_sb = cp.tile([128, 128], F32R, tag="perm")
            tri_sb = cp.tile([128, 128], F32, tag="tri")
            id_sb = cp.tile([128, 128], F32, tag="id")
            ones_sb = cp.tile([128, 1], F32R, tag="ones")
            nc.sync.dma_start(out=cos_sb[:], in_=cosc.ap())
            nc.sync.dma_start(out=sin_sb[:], in_=sinc.ap())
            nc.sync.dma_start(out=perm_sb[:], in_=permc.ap().bitcast(F32R))
            nc.sync.dma_start(out=tri_sb[:], in_=tric.ap())
            nc.sync.dma_start(out=id_sb[:], in_=identc.ap())
            nc.sync.dma_start(out=ones_sb[:], in_=onesc.ap().bitcast(F32R))

            tens = dict(
                mybir=mybir, F32=F32, F32R=F32R, rg=rg, sim=sim,
                xs=xs, wq=wq, wk=wk, wv=wv, wo=wo, y=y,
                cos_sb=cos_sb, sin_sb=sin_sb, perm_sb=perm_sb,
                tri_sb=tri_sb, id_sb=id_sb, ones_sb=ones_sb,
            )
            tens["phase_reps"] = phase_reps
            tens["dram"] = dram
            tens["until"] = until
            for rep in range(repeat):
                tens["xT_h"] = [
                    [dram.tile([HDQ, S // 2], F32R, name=f"xT_h{b}_{hf}_{rep}")
                     for hf in range(2)] for b in range(B)]
                tens["xT_F"] = [
                    [dram.tile([DM, S // 2], F32R, addr_space="Shared",
                               name=f"xT_F{b}_{hf}_{rep}") for hf in range(2)]
                    for b in range(B)]
                tens["oT_h"] = [dram.tile([HDQ, S], F32R, name=f"oT_h{b}_{rep}")
                                for b in range(B)]
                tens["oT_F"] = [dram.tile([DM, S], F32R, addr_space="Shared",
                                          name=f"oT_F{b}_{rep}")
                                for b in range(B)]
                _emit_body(nc, tc, tens, rep)

    nc.compile()
    return nc


def _emit_body(nc, tc, t, rep):
    mybir = t["mybir"]
    F32, F32R = t["F32"], t["F32R"]
    xs, wq, wk, wv, wo, y = t["xs"], t["wq"], t["wk"], t["wv"], t["wo"], t["y"]
    xT_h, xT_F, oT_h, oT_F = t["xT_h"], t["xT_F"], t["oT_h"], t["oT_F"]
    cos_sb, sin_sb = t["cos_sb"], t["sin_sb"]
    perm_sb, tri_sb, id_sb, ones_sb = (t["perm_sb"], t["tri_sb"], t["id_sb"],
                                       t["ones_sb"])
    rg, sim = t["rg"], t["sim"]
    until = t.get("until", "wo")
    lim = STAGES.index(until)

    def on(stage):
        return STAGES.index(stage) <= lim
    preps = t.get("phase_reps", (1, 1, 1, 1))
    if len(preps) == 4:
        preps = (*preps, 1)
    rp0, rqkv, rattn, rwo, rag = preps
    R = f"r{rep}"

    # ---------- phase 0: transpose own x dim-slice, AllGather x^T ----------
    with (
        tc.tile_pool(name=f"ps0{R}", bufs=2, space="PSUM") as ps0,
        tc.tile_pool(name=f"w0{R}", bufs=3) as wp,
    ):
        for _p0 in range(rp0):
          for b in range(B):
            for ttg in range(S // 512):
                hf, hcol = divmod(ttg, 2)  # groups of 4 token tiles
                x_tiles = []
                for j in range(4):
                    row = b * S + ttg * 512 + j * 128
                    x_t = wp.tile([128, HDQ], F32, tag=f"x_t{j}",
                                  name=f"x_t{j}")
                    nc.sync.dma_start(out=x_t[:], in_=xs.ap()[row:row + 128, :])
                    x_tiles.append(x_t)
                for dt_i in range(HDQ // 128):
                    pt = ps0.tile([128, 512], F32, tag="pt")
                    for j in range(4):
                        nc.tensor.transpose(
                            pt[:, j * 128:(j + 1) * 128],
                            x_tiles[j][:, dt_i * 128:(dt_i + 1) * 128],
                            id_sb[:],
                        )
                    xt_sb = wp.tile([128, 512], F32R, tag="xt_sb")
                    nc.scalar.copy(xt_sb[:], pt[:])
                    nc.sync.dma_start(
                        out=xT_h[b][hf][:][dt_i * 128:(dt_i + 1) * 128,
                                           hcol * 512:(hcol + 1) * 512],
                        in_=xt_sb[:],
                    )
                if not sim and on("xag") and _p0 == rp0 - 1 and hcol == 1:
                    nc.gpsimd.collective_compute(
                        "AllGather", mybir.AluOpType.bypass, replica_groups=rg,
                        ins=[xT_h[b][hf][:].opt()],
                        outs=[xT_F[b][hf][:].opt()],
                    )
        if not sim and on("xag"):
            for extra in range(rag - 1):
                for b in range(B):
                    dummy = t["dram"].tile(
                        [DM, S], F32R, addr_space="Shared",
                        name=f"xT_dummy{b}_{rep}_{extra}")
                    nc.gpsimd.collective_compute(
                        "AllGather", mybir.AluOpType.bypass, replica_groups=rg,
                        ins=[xT_h[b][0][:].opt(), ],
                        outs=[dummy[:][0:DM // 2, :].opt()],
                    )

    # ---------- weights (resident across both batches) ----------
    with tc.tile_pool(name=f"wqkv{R}", bufs=1) as wpool:
        wq_sb = wpool.tile([128, KC * HDQ], F32R, tag="wq")
        wk_sb = wpool.tile([128, KC * HD], F32R, tag="wk")
        wv_sb = wpool.tile([128, KC * HD], F32R, tag="wv")
        nc.sync.dma_start(
            out=wq_sb[:].rearrange("p (kc d) -> p kc d", kc=KC),
            in_=wq.ap().rearrange("(kc p) d -> p kc d", p=128).bitcast(F32R),
        )
        nc.sync.dma_start(
            out=wk_sb[:].rearrange("p (kc d) -> p kc d", kc=KC),
            in_=wk.ap().rearrange("(kc p) d -> p kc d", p=128).bitcast(F32R),
        )
        nc.sync.dma_start(
            out=wv_sb[:].rearrange("p (kc d) -> p kc d", kc=KC),
            in_=wv.ap().rearrange("(kc p) d -> p kc d", p=128).bitcast(F32R),
        )

        with tc.tile_pool(name=f"batch{R}", bufs=1) as bp:
            qT = [bp.tile([128, S], F32R, tag=f"qT{h}", name=f"qT{h}")
                  for h in range(NQH)]
            kT = bp.tile([128, S], F32R, tag="kT")
            v_nat = bp.tile([128, NKT * 128], F32R, tag="v_nat")

            for b in range(B):
              if not on(f"qkv{b}"):
                  continue
              for _rq in range(rqkv):
                # ---------- QKV projection (transposed form) ----------
                with (
                    tc.tile_pool(name=f"ps_acc{b}{R}q{_rq}", bufs=1,
                                 space="PSUM") as ps_acc,
                    tc.tile_pool(name=f"ps_rope{b}{R}q{_rq}", bufs=1,
                                 space="PSUM") as ps_rope,
                    tc.tile_pool(name=f"wqk{b}{R}q{_rq}", bufs=2) as wp,
                    tc.tile_pool(name=f"xtp{b}{R}q{_rq}", bufs=4) as xtp,
                ):
                    for tb in range(NTB):
                        tsl = slice(tb * TB, (tb + 1) * TB)
                        psq = [ps_acc.tile([128, TB], F32, tag=f"psq{i}",
                                           name=f"psq{i}") for i in range(NQH)]
                        psk = ps_acc.tile([128, TB], F32, tag="psk")
                        psv = ps_acc.tile([128, TB], F32, tag="psv")
                        for kc in range(KC):
                            xt_t = xtp.tile([128, TB], F32R, tag="xt_t")
                            hf, hcol = divmod(tb, 2)
                            hsl = slice(hcol * TB, (hcol + 1) * TB)
                            nc.sync.dma_start(
                                out=xt_t[:],
                                in_=xT_F[b][hf][:][kc * 128:(kc + 1) * 128,
                                                   hsl],
                            )
                            for i in range(NQH):
                                nc.tensor.matmul(
                                    psq[i][:],
                                    wq_sb[:, kc * HDQ + i * HD:
                                          kc * HDQ + (i + 1) * HD],
                                    xt_t[:],
                                    start=(kc == 0), stop=(kc == KC - 1),
                                )
                            nc.tensor.matmul(
                                psk[:], wk_sb[:, kc * HD:(kc + 1) * HD],
                                xt_t[:],
                                start=(kc == 0), stop=(kc == KC - 1),
                            )
                            nc.tensor.matmul(
                                psv[:], wv_sb[:, kc * HD:(kc + 1) * HD],
                                xt_t[:],
                                start=(kc == 0), stop=(kc == KC - 1),
                            )

                        cos_t = cos_sb[:, tsl]
                        sin_t = sin_sb[:, tsl]
                        for idx in range(NQH + 1):
                            acc = psq[idx] if idx < NQH else psk
                            dest = qT[idx][:] if idx < NQH else kT[:]
                            raw = wp.tile([128, TB], F32R, tag="rope_raw")
                            nc.scalar.copy(raw[:], acc[:])
                            swp = ps_rope.tile([128, TB], F32, tag="swp")
                            nc.tensor.matmul(swp[:], perm_sb[:], raw[:],
                                             start=True, stop=True)
                            t1 = wp.tile([128, TB], F32, tag="rope_t1")
                            nc.vector.tensor_mul(t1[:], raw[:].bitcast(F32),
                                                 cos_t)
                            t2 = wp.tile([128, TB], F32, tag="rope_t2")
                            nc.vector.tensor_mul(t2[:], swp[:], sin_t)
                            nc.vector.tensor_add(dest[:, tsl], t1[:], t2[:])

                        vt_sb = wp.tile([128, TB], F32, tag="vt_sb")
                        nc.scalar.copy(vt_sb[:], psv[:])
                        vp = ps_rope.tile([128, TB], F32, tag="vp")
                        for j in range(TB // 128):
                            nc.tensor.transpose(
                                vp[:, j * 128:(j + 1) * 128],
                                vt_sb[:, j * 128:(j + 1) * 128], id_sb[:])
                        nc.scalar.copy(
                            v_nat[:, tb * TB:(tb + 1) * TB], vp[:])

              for _ra in range(rattn if on(f"attn{b}") else 0):
                # ---------- attention ----------
                with (
                    tc.tile_pool(name=f"ps_s{b}{R}a{_ra}", bufs=3,
                                 space="PSUM") as ps_s,
                    tc.tile_pool(name=f"ps_o{b}{R}a{_ra}", bufs=2,
                                 space="PSUM") as ps_o,
                    tc.tile_pool(name=f"ps_sum{b}{R}a{_ra}", bufs=2,
                                 space="PSUM") as ps_sum,
                    tc.tile_pool(name=f"wa{b}{R}a{_ra}", bufs=2) as wp,
                    tc.tile_pool(name=f"ptp{b}{R}a{_ra}", bufs=3) as ptp,
                ):
                    for h in range(NQH):
                        for qb in range(NQB):
                            q0 = qb * QB
                            kt_max = (q0 + QB) // 128 - 1
                            sT = ps_s.tile([128, QB], F32, tag="sT")
                            oT = ps_o.tile([128, QB], F32, tag="oT")
                            sums = ps_sum.tile([1, QB], F32, tag="sums")
                            pT = ptp.tile([128, QB], F32R, tag="pT")
                            for kt in range(kt_max + 1):
                                off = max(0, kt * 128 - q0)
                                qs = slice(q0 + off, q0 + QB)
                                psl = slice(off, QB)
                                nc.tensor.matmul(
                                    sT[:, psl],
                                    kT[:, kt * 128:(kt + 1) * 128],
                                    qT[h][:, qs],
                                    start=True, stop=True,
                                )
                                nc.scalar.activation(
                                    pT[:, psl], sT[:, psl],
                                    mybir.ActivationFunctionType.Exp,
                                    scale=SCALE,
                                )
                                if kt * 128 >= q0:
                                    nc.vector.tensor_mul(
                                        pT[:, off:off + 128],
                                        pT[:, off:off + 128].bitcast(F32),
                                        tri_sb[:],
                                    )
                                nc.tensor.matmul(
                                    oT[:, psl],
                                    v_nat[:, kt * 128:(kt + 1) * 128],
                                    pT[:, psl],
                                    start=(kt == 0), stop=(kt == kt_max),
                                )
                                nc.tensor.matmul(
                                    sums[0:1, psl], ones_sb[:], pT[:, psl],
                                    start=(kt == 0), stop=(kt == kt_max),
                                )
                            sums_sb = wp.tile([1, QB], F32, tag="sums_sb")
                            nc.scalar.copy(sums_sb[:], sums[0:1, :])
                            rec = wp.tile([1, QB], F32, tag="rec")
                            scr = wp.tile([1, QB], F32, tag="scr")
                            nc.vector.reciprocal_approx_accurate(
                                rec[:], sums_sb[:], scr[:])
                            rb = wp.tile([128, QB], F32, tag="rb")
                            nc.gpsimd.partition_broadcast(rb[:], rec[:])
                            oT_sb = wp.tile([128, QB], F32R, tag="oT_sb")
                            nc.vector.tensor_mul(oT_sb[:], oT[:], rb[:])
                            nc.sync.dma_start(
                                out=oT_h[b][:][h * 128:(h + 1) * 128,
                                               q0:q0 + QB],
                                in_=oT_sb[:],
                            )
              if not sim and on(f"oag{b}"):
                nc.gpsimd.collective_compute(
                    "AllGather", mybir.AluOpType.bypass, replica_groups=rg,
                    ins=[oT_h[b][:].opt()], outs=[oT_F[b][:].opt()],
                )

    # ---------- WO projection (column shard) ----------
    if not on("wo"):
        dummy_y = t["dram"]  # noqa: F841
        nc.sync.dma_start(out=y.ap()[0:128, :], in_=cos_sb[:, 0:HDQ])
        return
    with tc.tile_pool(name=f"wo_p{R}", bufs=1) as wo_p:
        wo_sb = wo_p.tile([128, KC * HDQ], F32R, tag="wo")
        nc.sync.dma_start(
            out=wo_sb[:].rearrange("p (kc d) -> p kc d", kc=KC),
            in_=wo.ap().rearrange("(kc p) d -> p kc d", p=128).bitcast(F32R),
        )
        for _rw in range(rwo):
          with (
            tc.tile_pool(name=f"ps_y{R}w{_rw}", bufs=2, space="PSUM") as ps_y,
            tc.tile_pool(name=f"w_wo{R}w{_rw}", bufs=2) as wp,
            tc.tile_pool(name=f"otp{R}w{_rw}", bufs=3) as otp,
          ):
            for b in range(B):
                for tt in range(S // 128):
                    ot_strip = otp.tile([128, KC * 128], F32R, tag="ot_strip")
                    nc.sync.dma_start(
                        out=ot_strip[:].rearrange("p (hc t) -> p hc t", hc=KC),
                        in_=oT_F[b][:]
                        .rearrange("(hc p) t -> p hc t", p=128)
                        [:, :, tt * 128:(tt + 1) * 128],
                    )
                    psy = ps_y.tile([128, HDQ], F32, tag="psy")
                    for hc in range(KC):
                        nc.tensor.matmul(
                            psy[:],
                            ot_strip[:, hc * 128:(hc + 1) * 128],
                            wo_sb[:, hc * HDQ:(hc + 1) * HDQ],
                            start=(hc == 0), stop=(hc == KC - 1),
                        )
                    y_sb = wp.tile([128, HDQ], F32, tag="y_sb")
                    nc.scalar.copy(y_sb[:], psy[:])
                    row = b * S + tt * 128
                    nc.sync.dma_start(out=y.ap()[row:row + 128, :], in_=y_sb[:])


def _in_maps(x, wq, wk, wv, wo):
    x2 = np.asarray(x, dtype=np.float32).reshape(T, DM)
    cosT, sinT, perm, tri, ident, ones = _consts()
    maps = []
    for c in range(N_CORES):
        qsl = slice(c * HDQ, (c + 1) * HDQ)
        ksl = slice(c * HD, (c + 1) * HD)
        maps.append({
            "xs": np.ascontiguousarray(x2[:, qsl]),
            "wq": np.ascontiguousarray(np.asarray(wq, np.float32)[:, qsl]),
            "wk": np.ascontiguousarray(np.asarray(wk, np.float32)[:, ksl]),
            "wv": np.ascontiguousarray(np.asarray(wv, np.float32)[:, ksl]),
            "wo": np.ascontiguousarray(np.asarray(wo, np.float32)[:, qsl]),
            "cosc": cosT, "sinc": sinT, "permc": perm, "tric": tri,
            "identc": ident, "onesc": ones,
        })
    return maps


def kernel(x, wq, wk, wv, wo, start_pos=0, **_unused):
    from concourse import bass_utils

    assert int(np.asarray(start_pos)) == 0
    in_maps = _in_maps(x, wq, wk, wv, wo)

    if "nc" not in _CACHE:
        _CACHE["nc"] = _build()
    nc = _CACHE["nc"]

    res = bass_utils.run_bass_kernel_spmd(
        nc, in_maps, core_ids=list(range(N_CORES)),
        trace=bool(int(os.environ.get("KERNEL_TRACE", "0") or 0)),
    )
    _CACHE["last_result"] = res

    out = np.empty((T, DM), np.float32)
    for c in range(N_CORES):
        out[:, c * HDQ:(c + 1) * HDQ] = res.results[c]["y"]
    return out.reshape(B, S, DM)



# revision 6
# speedup vs baseline: 1.5059x; 1.5059x over previous
"""Trainium2 Bass kernel for nn_Attention_33354716021131 (v2).

Dense GQA attention (B=2, S=2048, D=4096, 32 q-heads / 8 kv-heads, head_dim
128, RoPE, causal softmax) tensor-parallel across 8 NeuronCores.

Per core c: q-heads 4c..4c+3 (kv-head c) -> wq/wk/wv column shards, wo column
shard; host passes x pre-transposed (xT [D, T], bf16) to every core, so there
is no on-device input transpose and no input collective.  The only collectives
are two AllGathers (one per batch) of the attention outputs oT (bf16).

Pipeline per core:
  QKV   x-stationary matmuls produce q/k/v in natural [token, feat] layout
        (256-token granules, PSUM: 2x q-bank + 2x kv-bank), RoPE applied on
        the free axis with plain DVE ops, then q/k are PE-transposed into
        qT/kT [d, token]; v stays natural.  bf16 inputs, fp32 PSUM.
  ATTN  per (head, 512-query block): sT = kT_tile^T qT (fp32r), pT =
        exp(sT*scale) on ScalarE, causal tri-mask on diagonal tiles, oT +=
        v_nat_tile^T pT, denominators via ones-matmul; normalize with DVE
        reciprocal + partition-broadcast multiply (GpSimd only runs the two
        collectives).  Output oT written bf16.
  AG    AllGather oT [512, 2048] -> oT_F [4096, 2048] per batch (bf16).
  WO    strip-stationary: load oT_F row-strips [128 f, 512 t] (contiguous 1KB
        lines), psy[tti] += strip_chunk^T wo_chunk accumulated over 32 feature
        chunks; 4 token-tiles per group, PSUM double-buffered (8 banks).
All matmuls run at 1 cycle/row (bf16 or fp32r with free >= 256).
"""
import math
import os

import numpy as np

N_CORES = 8
B = 2
S = 2048
DM = 4096
N_HEADS = 32
HD = 128
NQH = N_HEADS // N_CORES          # 4 q heads per core
HDQ = NQH * HD                    # 512
T = B * S                         # 4096 tokens
KC = DM // 128                    # 32 contraction chunks
NG = S // 256                     # 8 granules (256 tokens) per batch
NGT = S // 128                    # 16 token tiles per batch
QB = 512                          # query block for attention
NQB = S // QB                     # 4
SCALE = 1.0 / math.sqrt(HD)
ROPE_THETA = 10000.0

_CACHE = {}


def _consts():
    j = np.arange(HD // 2)
    inv = 1.0 / (ROPE_THETA ** (2 * j / HD))          # [64]
    pos = np.arange(S).reshape(NGT, 128)              # [16, 128]
    ang = pos[:, :, None] * inv[None, None, :]        # [16, 128, 64]
    cos = np.cos(ang).astype(np.float32)
    sin = np.sin(ang).astype(np.float32)
    # [128 part, 16 tiles, 4 head-reps, 64 freqs] -> [128, 4096]
    cos4 = np.tile(cos.transpose(1, 0, 2)[:, :, None, :], (1, 1, NQH, 1))
    sin4 = np.tile(sin.transpose(1, 0, 2)[:, :, None, :], (1, 1, NQH, 1))
    cos4 = np.ascontiguousarray(cos4.reshape(128, NGT * NQH * 64))
    sin4 = np.ascontiguousarray(sin4.reshape(128, NGT * NQH * 64))
    tri = (np.arange(128)[:, None] <= np.arange(128)[None, :]).astype(np.float32)
    ident = np.eye(128, dtype=np.float32)
    ones = np.ones((128, 1), np.float32)
    return cos4, sin4, tri, ident, ones


def _build(sim=False):
    import concourse.mybir as mybir
    import concourse.tile as tile
    from concourse import bacc

    F32 = mybir.dt.float32
    F32R = mybir.dt.float32r
    BF16 = mybir.dt.bfloat16

    nc = bacc.Bacc("TRN2", target_bir_lowering=False, debug=False,
                   num_devices=N_CORES)

    xt = nc.dram_tensor("xt", [DM, T], BF16, kind="ExternalInput")
    wq = nc.dram_tensor("wq", [DM, HDQ], BF16, kind="ExternalInput")
    wkv = nc.dram_tensor("wkv", [DM, 256], BF16, kind="ExternalInput")
    wo = nc.dram_tensor("wo", [DM, HDQ], BF16, kind="ExternalInput")
    cosc = nc.dram_tensor("cosc", [128, NGT * 256], F32, kind="ExternalInput")
    sinc = nc.dram_tensor("sinc", [128, NGT * 256], F32, kind="ExternalInput")
    tric = nc.dram_tensor("tric", [128, 128], F32, kind="ExternalInput")
    identc = nc.dram_tensor("identc", [128, 128], F32, kind="ExternalInput")
    onesc = nc.dram_tensor("onesc", [128, 1], F32, kind="ExternalInput")

    y = nc.dram_tensor("y", [T, HDQ], F32, kind="ExternalOutput")

    rg = [list(range(N_CORES))]

    with tile.TileContext(nc) as tc:
        with (
            tc.tile_pool(name="dram", bufs=1, space="DRAM") as dram,
            tc.tile_pool(name="const", bufs=1) as cp,
        ):
            cos_sb = cp.tile([128, NGT * 256], F32, tag="cos")
            sin_sb = cp.tile([128, NGT * 256], F32, tag="sin")
            tri_sb = cp.tile([128, 128], F32, tag="tri")
            id_sb = cp.tile([128, 128], F32R, tag="id")
            ones_sb = cp.tile([128, 1], F32R, tag="ones")
            nc.sync.dma_start(out=cos_sb[:], in_=cosc.ap())
            nc.sync.dma_start(out=sin_sb[:], in_=sinc.ap())
            nc.sync.dma_start(out=tri_sb[:], in_=tric.ap())
            nc.sync.dma_start(out=id_sb[:], in_=identc.ap().bitcast(F32R))
            nc.sync.dma_start(out=ones_sb[:], in_=onesc.ap().bitcast(F32R))

            oT_h = [dram.tile([HDQ, S], BF16, name=f"oT_h{b}") for b in range(B)]
            oT_F = [dram.tile([DM, S], BF16,
                              addr_space="Local" if sim else "Shared",
                              name=f"oT_F{b}") for b in range(B)]

            with tc.tile_pool(name="wqkv", bufs=1) as wpool:
                wq_sb = wpool.tile([128, KC * HDQ], BF16, tag="wq")
                wkv_sb = wpool.tile([128, KC * 256], BF16, tag="wkv")
                nc.sync.dma_start(
                    out=wq_sb[:].rearrange("p (kc d) -> p kc d", kc=KC),
                    in_=wq.ap().rearrange("(kc p) d -> p kc d", p=128),
                )
                nc.sync.dma_start(
                    out=wkv_sb[:].rearrange("p (kc d) -> p kc d", kc=KC),
                    in_=wkv.ap().rearrange("(kc p) d -> p kc d", p=128),
                )

                with tc.tile_pool(name="batch", bufs=1) as bp:
                    qTall = bp.tile([128, NQH * S], F32R, tag="qTall")
                    kT = bp.tile([128, S], F32R, tag="kT")
                    v_nat = bp.tile([128, S], F32R, tag="v_nat")

                    for b in range(B):
                        _emit_qkv(nc, tc, b, dict(
                            mybir=mybir, F32=F32, F32R=F32R, BF16=BF16,
                            xt=xt, wq_sb=wq_sb, wkv_sb=wkv_sb,
                            cos_sb=cos_sb, sin_sb=sin_sb, id_sb=id_sb,
                            qTall=qTall, kT=kT, v_nat=v_nat,
                        ))
                        _emit_attn(nc, tc, b, dict(
                            mybir=mybir, F32=F32, F32R=F32R, BF16=BF16,
                            qTall=qTall, kT=kT, v_nat=v_nat,
                            tri_sb=tri_sb, ones_sb=ones_sb, oT_h=oT_h,
                        ))
                        if not sim:
                            nc.gpsimd.collective_compute(
                                "AllGather", mybir.AluOpType.bypass,
                                replica_groups=rg,
                                ins=[oT_h[b][:].opt()],
                                outs=[oT_F[b][:].opt()],
                            )
                        else:
                            # sim: fake the gather by replicating the local
                            # shard into every rank slot of oT_F.
                            for c in range(N_CORES):
                                nc.sync.dma_start(
                                    out=oT_F[b][:][c * HDQ:(c + 1) * HDQ, :],
                                    in_=oT_h[b][:],
                                )

            # ---------- WO projection ----------
            with (
                tc.tile_pool(name="wo_p", bufs=1) as wo_p,
                tc.tile_pool(name="ps_y", bufs=2, space="PSUM") as ps_y,
                tc.tile_pool(name="stp", bufs=4) as stp,
                tc.tile_pool(name="ywp", bufs=2) as ywp,
            ):
                wo_sb = wo_p.tile([128, KC * HDQ], BF16, tag="wo")
                nc.sync.dma_start(
                    out=wo_sb[:].rearrange("p (kc d) -> p kc d", kc=KC),
                    in_=wo.ap().rearrange("(kc p) d -> p kc d", p=128),
                )
                for b in range(B):
                    for tg in range(S // 512):
                        psy = [ps_y.tile([128, HDQ], F32, tag=f"psy{i}",
                                         name=f"psy{i}") for i in range(4)]
                        for hc in range(KC):
                            strip = stp.tile([128, 512], BF16, tag="strip")
                            nc.sync.dma_start(
                                out=strip[:],
                                in_=oT_F[b][:][hc * 128:(hc + 1) * 128,
                                               tg * 512:(tg + 1) * 512],
                            )
                            for tti in range(4):
                                nc.tensor.matmul(
                                    psy[tti][:],
                                    strip[:, tti * 128:(tti + 1) * 128],
                                    wo_sb[:, hc * HDQ:(hc + 1) * HDQ],
                                    start=(hc == 0), stop=(hc == KC - 1),
                                )
                        for tti in range(4):
                            y_sb = ywp.tile([128, HDQ], F32, tag="y_sb")
                            nc.scalar.copy(y_sb[:], psy[tti][:])
                            row = b * S + tg * 512 + tti * 128
                            nc.scalar.dma_start(out=y.ap()[row:row + 128, :],
                                                in_=y_sb[:])

    nc.compile()
    return nc


def _emit_qkv(nc, tc, b, t):
    F32, F32R, BF16 = t["F32"], t["F32R"], t["BF16"]
    xt, wq_sb, wkv_sb = t["xt"], t["wq_sb"], t["wkv_sb"]
    cos_sb, sin_sb, id_sb = t["cos_sb"], t["sin_sb"], t["id_sb"]
    qTall, kT, v_nat = t["qTall"], t["kT"], t["v_nat"]

    with (
        tc.tile_pool(name=f"ps_acc{b}", bufs=1, space="PSUM") as ps_acc,
        tc.tile_pool(name=f"ps_T{b}", bufs=1, space="PSUM") as ps_T,
        tc.tile_pool(name=f"xtp{b}", bufs=4) as xtp,
        tc.tile_pool(name=f"rwp{b}", bufs=2) as rwp,
        tc.tile_pool(name=f"qrp{b}", bufs=2) as qrp,
        tc.tile_pool(name=f"tmp{b}", bufs=2) as tmp,
    ):
        def emit_T(prev):
            if prev is None:
                return
            g0, q_rots, k_rots = prev
            pos = g0 * 256
            for tti in range(2):
                tq = ps_T.tile([128, HDQ], F32R, tag=f"tq{tti}",
                               name=f"tq{tti}")
                for h in range(NQH):
                    nc.tensor.transpose(
                        tq[:, h * 128:(h + 1) * 128],
                        q_rots[tti][:, h * 128:(h + 1) * 128],
                        id_sb[:],
                    )
                nc.scalar.copy(
                    qTall[:].rearrange("p (h s) -> p h s", h=NQH)
                    [:, :, pos + tti * 128:pos + (tti + 1) * 128],
                    tq[:].rearrange("p (h t) -> p h t", h=NQH),
                )
            tk = ps_T.tile([128, 256], F32R, tag="tk",
                           padded_shape=[128, 512])
            for tti in range(2):
                nc.tensor.transpose(
                    tk[:, tti * 128:(tti + 1) * 128],
                    k_rots[tti][:],
                    id_sb[:],
                )
            nc.scalar.copy(kT[:, pos:pos + 256], tk[:])

        prev = None
        for g in range(NG):
            tok0 = b * S + g * 256
            psq = [ps_acc.tile([128, HDQ], F32, tag=f"psq{i}", name=f"psq{i}")
                   for i in range(2)]
            pskv = [ps_acc.tile([128, 256], F32, tag=f"pskv{i}",
                                name=f"pskv{i}", padded_shape=[128, 512])
                    for i in range(2)]
            for kcp in range(KC // 2):
                xt2 = xtp.tile([128, 512], BF16, tag="xt2")
                nc.sync.dma_start(
                    out=xt2[:].rearrange("p (c t) -> p c t", c=2),
                    in_=xt.ap()[kcp * 256:(kcp + 1) * 256, tok0:tok0 + 256]
                    .rearrange("(c p) t -> p c t", p=128),
                )
                for c2 in range(2):
                    kc = kcp * 2 + c2
                    for tti in range(2):
                        lhsT = xt2[:, c2 * 256 + tti * 128:
                                   c2 * 256 + (tti + 1) * 128]
                        nc.tensor.matmul(
                            psq[tti][:], lhsT,
                            wq_sb[:, kc * HDQ:(kc + 1) * HDQ],
                            start=(kc == 0), stop=(kc == KC - 1),
                        )
                        nc.tensor.matmul(
                            pskv[tti][:], lhsT,
                            wkv_sb[:, kc * 256:(kc + 1) * 256],
                            start=(kc == 0), stop=(kc == KC - 1),
                        )

            emit_T(prev)

            q_rots, k_rots = [], []
            for tti in range(2):
                gt = g * 2 + tti
                rq = rwp.tile([128, HDQ], F32, tag=f"rq{tti}", name=f"rq{tti}")
                nc.scalar.copy(rq[:], psq[tti][:])
                rkv = rwp.tile([128, 256], F32, tag=f"rkv{tti}",
                               name=f"rkv{tti}")
                nc.scalar.copy(rkv[:], pskv[tti][:])
                nc.scalar.copy(v_nat[:, gt * 128:(gt + 1) * 128],
                               rkv[:, 128:256])

                csl = slice(gt * 256, gt * 256 + 256)
                ksl = slice(gt * 256, gt * 256 + 64)
                q_rot = qrp.tile([128, HDQ], F32R, tag=f"qr{tti}",
                                 name=f"qr{tti}")
                x0 = rq[:].rearrange("p (d two) -> p d two", two=2)[:, :, 0]
                x1 = rq[:].rearrange("p (d two) -> p d two", two=2)[:, :, 1]
                r0 = q_rot[:].rearrange("p (d two) -> p d two", two=2)[:, :, 0]
                r1 = q_rot[:].rearrange("p (d two) -> p d two", two=2)[:, :, 1]
                m0 = tmp.tile([128, 256], F32, tag="m0", name="m0")
                m1 = tmp.tile([128, 256], F32, tag="m1", name="m1")
                nc.vector.tensor_mul(m0[:], x0, cos_sb[:, csl])
                nc.vector.tensor_mul(m1[:], x1, sin_sb[:, csl])
                nc.vector.tensor_sub(r0, m0[:], m1[:])
                m2 = tmp.tile([128, 256], F32, tag="m0", name="m2")
                m3 = tmp.tile([128, 256], F32, tag="m1", name="m3")
                nc.vector.tensor_mul(m2[:], x0, sin_sb[:, csl])
                nc.vector.tensor_mul(m3[:], x1, cos_sb[:, csl])
                nc.vector.tensor_add(r1, m2[:], m3[:])

                k_rot = qrp.tile([128, 128], F32R, tag=f"kr{tti}",
                                 name=f"kr{tti}")
                kx0 = rkv[:, 0:128].rearrange("p (d two) -> p d two",
                                              two=2)[:, :, 0]
                kx1 = rkv[:, 0:128].rearrange("p (d two) -> p d two",
                                              two=2)[:, :, 1]
                kr0 = k_rot[:].rearrange("p (d two) -> p d two",
                                         two=2)[:, :, 0]
                kr1 = k_rot[:].rearrange("p (d two) -> p d two",
                                         two=2)[:, :, 1]
                km0 = tmp.tile([128, 64], F32, tag="km0", name="km0")
                km1 = tmp.tile([128, 64], F32, tag="km1", name="km1")
                nc.vector.tensor_mul(km0[:], kx0, cos_sb[:, ksl])
                nc.vector.tensor_mul(km1[:], kx1, sin_sb[:, ksl])
                nc.vector.tensor_sub(kr0, km0[:], km1[:])
                km2 = tmp.tile([128, 64], F32, tag="km0", name="km2")
                km3 = tmp.tile([128, 64], F32, tag="km1", name="km3")
                nc.vector.tensor_mul(km2[:], kx0, sin_sb[:, ksl])
                nc.vector.tensor_mul(km3[:], kx1, cos_sb[:, ksl])
                nc.vector.tensor_add(kr1, km2[:], km3[:])
                q_rots.append(q_rot)
                k_rots.append(k_rot)

            prev = (g, q_rots, k_rots)
        emit_T(prev)


def _emit_attn(nc, tc, b, t):
    mybir = t["mybir"]
    F32, F32R, BF16 = t["F32"], t["F32R"], t["BF16"]
    qTall, kT, v_nat = t["qTall"], t["kT"], t["v_nat"]
    tri_sb, ones_sb, oT_h = t["tri_sb"], t["ones_sb"], t["oT_h"]

    with (
        tc.tile_pool(name=f"ps_s{b}", bufs=3, space="PSUM") as ps_s,
        tc.tile_pool(name=f"ps_o{b}", bufs=3, space="PSUM") as ps_o,
        tc.tile_pool(name=f"ps_sum{b}", bufs=2, space="PSUM") as ps_sum,
        tc.tile_pool(name=f"wa{b}", bufs=2) as wp,
        tc.tile_pool(name=f"ptp{b}", bufs=3) as ptp,
    ):
        for h in range(NQH):
            for qb in range(NQB):
                q0 = qb * QB
                kt_max = (q0 + QB) // 128 - 1
                sT = ps_s.tile([128, QB], F32, tag="sT")
                oT = ps_o.tile([128, QB], F32, tag="oT")
                sums = ps_sum.tile([1, QB], F32, tag="sums")
                pT = ptp.tile([128, QB], F32R, tag="pT")
                for kt in range(kt_max + 1):
                    off = max(0, kt * 128 - q0)
                    qs = slice(h * S + q0 + off, h * S + q0 + QB)
                    psl = slice(off, QB)
                    nc.tensor.matmul(
                        sT[:, psl],
                        kT[:, kt * 128:(kt + 1) * 128],
                        qTall[:, qs],
                        start=True, stop=True,
                    )
                    nc.scalar.activation(
                        pT[:, psl], sT[:, psl],
                        mybir.ActivationFunctionType.Exp,
                        scale=SCALE,
                    )
                    if kt * 128 >= q0:
                        nc.vector.tensor_mul(
                            pT[:, off:off + 128],
                            pT[:, off:off + 128].bitcast(F32),
                            tri_sb[:],
                        )
                    nc.tensor.matmul(
                        oT[:, psl],
                        v_nat[:, kt * 128:(kt + 1) * 128],
                        pT[:, psl],
                        start=(kt == 0), stop=(kt == kt_max),
                    )
                    nc.tensor.matmul(
                        sums[0:1, psl], ones_sb[:], pT[:, psl],
                        start=(kt == 0), stop=(kt == kt_max),
                    )
                sums_sb = wp.tile([1, QB], F32, tag="sums_sb")
                nc.scalar.copy(sums_sb[:], sums[0:1, :])
                rec = wp.tile([1, QB], F32, tag="rec")
                scr = wp.tile([1, QB], F32, tag="scr")
                nc.vector.reciprocal_approx_accurate(rec[:], sums_sb[:],
                                                     scr[:])
                rb = wp.tile([128, QB], F32, tag="rb")
                nc.gpsimd.partition_broadcast(rb[:], rec[:])
                oT_sb = wp.tile([128, QB], BF16, tag="oT_sb")
                nc.vector.tensor_mul(oT_sb[:], oT[:], rb[:])
                nc.scalar.dma_start(
                    out=oT_h[b][:][h * 128:(h + 1) * 128, q0:q0 + QB],
                    in_=oT_sb[:],
                )


def _in_maps(x, wq, wk, wv, wo):
    import concourse.mybir as mybir
    np_bf16 = mybir.dt.np(mybir.dt.bfloat16)

    x2 = np.asarray(x, dtype=np.float32).reshape(T, DM)
    xT = np.ascontiguousarray(x2.T).astype(np_bf16)
    cos4, sin4, tri, ident, ones = _consts()
    wq = np.asarray(wq, np.float32)
    wk = np.asarray(wk, np.float32)
    wv = np.asarray(wv, np.float32)
    wo = np.asarray(wo, np.float32)
    maps = []
    for c in range(N_CORES):
        qsl = slice(c * HDQ, (c + 1) * HDQ)
        ksl = slice(c * HD, (c + 1) * HD)
        wkv_c = np.concatenate([wk[:, ksl], wv[:, ksl]], axis=1)
        maps.append({
            "xt": xT,
            "wq": np.ascontiguousarray(wq[:, qsl]).astype(np_bf16),
            "wkv": np.ascontiguousarray(wkv_c).astype(np_bf16),
            "wo": np.ascontiguousarray(wo[:, qsl]).astype(np_bf16),
            "cosc": cos4, "sinc": sin4, "tric": tri,
            "identc": ident, "onesc": ones,
        })
    return maps


def kernel(x, wq, wk, wv, wo, start_pos=0, **_unused):
    from concourse import bass_utils

    assert int(np.asarray(start_pos)) == 0
    in_maps = _in_maps(x, wq, wk, wv, wo)

    if "nc" not in _CACHE:
        _CACHE["nc"] = _build()
    nc = _CACHE["nc"]

    res = bass_utils.run_bass_kernel_spmd(
        nc, in_maps, core_ids=list(range(N_CORES)),
        trace=bool(int(os.environ.get("KERNEL_TRACE", "0") or 0)),
    )
    _CACHE["last_result"] = res

    out = np.empty((T, DM), np.float32)
    for c in range(N_CORES):
        out[:, c * HDQ:(c + 1) * HDQ] = res.results[c]["y"]
    return out.reshape(B, S, DM)


# revision 8
# speedup vs baseline: 1.6320x; 1.0837x over previous
"""Trainium2 Bass kernel for nn_Attention_33354716021131 (v2).

Dense GQA attention (B=2, S=2048, D=4096, 32 q-heads / 8 kv-heads, head_dim
128, RoPE, causal softmax) tensor-parallel across 8 NeuronCores.

Per core c: q-heads 4c..4c+3 (kv-head c) -> wq/wk/wv column shards, wo column
shard; host passes x pre-transposed (xT [D, T], bf16) to every core, so there
is no on-device input transpose and no input collective.  The only collectives
are two AllGathers (one per batch) of the attention outputs oT (bf16).

Pipeline per core:
  QKV   x-stationary matmuls produce q/k/v in natural [token, feat] layout
        (256-token granules, PSUM: 2x q-bank + 2x kv-bank), RoPE applied on
        the free axis with plain DVE ops, then q/k are PE-transposed into
        qT/kT [d, token]; v stays natural.  bf16 inputs, fp32 PSUM.
  ATTN  per (head, 512-query block): sT = kT_tile^T qT (fp32r), pT =
        exp(sT*scale) on ScalarE, causal tri-mask on diagonal tiles, oT +=
        v_nat_tile^T pT, denominators via ones-matmul; normalize with DVE
        reciprocal + partition-broadcast multiply (GpSimd only runs the two
        collectives).  Output oT written bf16.
  AG    AllGather oT [512, 2048] -> oT_F [4096, 2048] per batch (bf16).
  WO    strip-stationary: load oT_F row-strips [128 f, 512 t] (contiguous 1KB
        lines), psy[tti] += strip_chunk^T wo_chunk accumulated over 32 feature
        chunks; 4 token-tiles per group, PSUM double-buffered (8 banks).
All matmuls run at 1 cycle/row (bf16 or fp32r with free >= 256).
"""
import math
import os

import numpy as np

N_CORES = 8
B = 2
S = 2048
DM = 4096
N_HEADS = 32
HD = 128
NQH = N_HEADS // N_CORES          # 4 q heads per core
HDQ = NQH * HD                    # 512
T = B * S                         # 4096 tokens
KC = DM // 128                    # 32 contraction chunks
NG = S // 256                     # 8 granules (256 tokens) per batch
NGT = S // 128                    # 16 token tiles per batch
QB = 512                          # query block for attention
NQB = S // QB                     # 4
SCALE = 1.0 / math.sqrt(HD)
ROPE_THETA = 10000.0

_CACHE = {}


def _consts():
    j = np.arange(HD // 2)
    inv = 1.0 / (ROPE_THETA ** (2 * j / HD))          # [64]
    pos = np.arange(S).reshape(NGT, 128)              # [16, 128]
    ang = pos[:, :, None] * inv[None, None, :]        # [16, 128, 64]
    cos = np.cos(ang).astype(np.float32)
    sin = np.sin(ang).astype(np.float32)
    # [128 part, 16 tiles, 4 head-reps, 64 freqs] -> [128, 4096]
    cos4 = np.tile(cos.transpose(1, 0, 2)[:, :, None, :], (1, 1, NQH, 1))
    sin4 = np.tile(sin.transpose(1, 0, 2)[:, :, None, :], (1, 1, NQH, 1))
    cos4 = np.ascontiguousarray(cos4.reshape(128, NGT * NQH * 64))
    sin4 = np.ascontiguousarray(sin4.reshape(128, NGT * NQH * 64))
    tri = (np.arange(128)[:, None] <= np.arange(128)[None, :]).astype(np.float32)
    ident = np.eye(128, dtype=np.float32)
    ones = np.ones((128, 1), np.float32)
    return cos4, sin4, tri, ident, ones


def _build(sim=False):
    import concourse.mybir as mybir
    import concourse.tile as tile
    from concourse import bacc

    F32 = mybir.dt.float32
    F32R = mybir.dt.float32r
    BF16 = mybir.dt.bfloat16

    nc = bacc.Bacc("TRN2", target_bir_lowering=False, debug=False,
                   num_devices=N_CORES)

    xt = nc.dram_tensor("xt", [DM, T], BF16, kind="ExternalInput")
    wq = nc.dram_tensor("wq", [DM, HDQ], BF16, kind="ExternalInput")
    wkv = nc.dram_tensor("wkv", [DM, 256], BF16, kind="ExternalInput")
    wo = nc.dram_tensor("wo", [DM, HDQ], BF16, kind="ExternalInput")
    cosc = nc.dram_tensor("cosc", [128, NGT * 256], F32, kind="ExternalInput")
    sinc = nc.dram_tensor("sinc", [128, NGT * 256], F32, kind="ExternalInput")
    tric = nc.dram_tensor("tric", [128, 128], F32, kind="ExternalInput")
    identc = nc.dram_tensor("identc", [128, 128], F32, kind="ExternalInput")
    onesc = nc.dram_tensor("onesc", [128, 1], F32, kind="ExternalInput")

    y = nc.dram_tensor("y", [T, HDQ], F32, kind="ExternalOutput")

    rg = [list(range(N_CORES))]

    with tile.TileContext(nc) as tc:
        with (
            tc.tile_pool(name="dram", bufs=1, space="DRAM") as dram,
            tc.tile_pool(name="const", bufs=1) as cp,
        ):
            cos_sb = cp.tile([128, NGT * 256], F32, tag="cos")
            sin_sb = cp.tile([128, NGT * 256], F32, tag="sin")
            tri_sb = cp.tile([128, 128], F32, tag="tri")
            id_sb = cp.tile([128, 128], F32R, tag="id")
            ones_sb = cp.tile([128, 1], F32R, tag="ones")
            nc.sync.dma_start(out=cos_sb[:], in_=cosc.ap())
            nc.sync.dma_start(out=sin_sb[:], in_=sinc.ap())
            nc.sync.dma_start(out=tri_sb[:], in_=tric.ap())
            nc.sync.dma_start(out=id_sb[:], in_=identc.ap().bitcast(F32R))
            nc.sync.dma_start(out=ones_sb[:], in_=onesc.ap().bitcast(F32R))

            oT_h = [dram.tile([HDQ, S], BF16, name=f"oT_h{b}") for b in range(B)]
            oT_F = [dram.tile([DM, S], BF16,
                              addr_space="Local" if sim else "Shared",
                              name=f"oT_F{b}") for b in range(B)]

            with tc.tile_pool(name="wqkv", bufs=1) as wpool:
                wq_sb = wpool.tile([128, KC * HDQ], BF16, tag="wq")
                wkv_sb = wpool.tile([128, KC * 256], BF16, tag="wkv")
                nc.sync.dma_start(
                    out=wq_sb[:].rearrange("p (kc d) -> p kc d", kc=KC),
                    in_=wq.ap().rearrange("(kc p) d -> p kc d", p=128),
                )
                nc.sync.dma_start(
                    out=wkv_sb[:].rearrange("p (kc d) -> p kc d", kc=KC),
                    in_=wkv.ap().rearrange("(kc p) d -> p kc d", p=128),
                )
                wo_sb = wpool.tile([128, KC * HDQ], BF16, tag="wo")
                nc.scalar.dma_start(
                    out=wo_sb[:].rearrange("p (kc d) -> p kc d", kc=KC),
                    in_=wo.ap().rearrange("(kc p) d -> p kc d", p=128),
                )

                with tc.tile_pool(name="batch", bufs=1) as bp:
                    qTall = bp.tile([128, NQH * S], F32R, tag="qTall")
                    kT = bp.tile([128, S], F32R, tag="kT")
                    v_nat = bp.tile([128, S], F32R, tag="v_nat")

                    for b in range(B):
                        _emit_qkv(nc, tc, b, dict(
                            mybir=mybir, F32=F32, F32R=F32R, BF16=BF16,
                            xt=xt, wq_sb=wq_sb, wkv_sb=wkv_sb,
                            cos_sb=cos_sb, sin_sb=sin_sb, id_sb=id_sb,
                            qTall=qTall, kT=kT, v_nat=v_nat,
                        ))
                        _emit_attn(nc, tc, b, dict(
                            mybir=mybir, F32=F32, F32R=F32R, BF16=BF16,
                            qTall=qTall, kT=kT, v_nat=v_nat,
                            tri_sb=tri_sb, ones_sb=ones_sb, oT_h=oT_h,
                        ))
                        if not sim:
                            nc.gpsimd.collective_compute(
                                "AllGather", mybir.AluOpType.bypass,
                                replica_groups=rg,
                                ins=[oT_h[b][:].opt()],
                                outs=[oT_F[b][:].opt()],
                            )
                        else:
                            # sim: fake the gather by replicating the local
                            # shard into every rank slot of oT_F.
                            for c in range(N_CORES):
                                nc.sync.dma_start(
                                    out=oT_F[b][:][c * HDQ:(c + 1) * HDQ, :],
                                    in_=oT_h[b][:],
                                )

                # ---------- WO projection ----------
                with (
                    tc.tile_pool(name="ps_y", bufs=2, space="PSUM") as ps_y,
                    tc.tile_pool(name="stp", bufs=4) as stp,
                    tc.tile_pool(name="ywp", bufs=2) as ywp,
                ):
                  for b in range(B):
                    for tg in range(S // 512):
                        psy = [ps_y.tile([128, HDQ], F32, tag=f"psy{i}",
                                         name=f"psy{i}") for i in range(4)]
                        for hc in range(KC):
                            strip = stp.tile([128, 512], BF16, tag="strip")
                            seng = nc.sync if hc % 2 == 0 else nc.scalar
                            seng.dma_start(
                                out=strip[:],
                                in_=oT_F[b][:][hc * 128:(hc + 1) * 128,
                                               tg * 512:(tg + 1) * 512],
                            )
                            for tti in range(4):
                                nc.tensor.matmul(
                                    psy[tti][:],
                                    strip[:, tti * 128:(tti + 1) * 128],
                                    wo_sb[:, hc * HDQ:(hc + 1) * HDQ],
                                    start=(hc == 0), stop=(hc == KC - 1),
                                )
                        for tti in range(4):
                            y_sb = ywp.tile([128, HDQ], F32, tag="y_sb")
                            nc.scalar.copy(y_sb[:], psy[tti][:])
                            row = b * S + tg * 512 + tti * 128
                            nc.scalar.dma_start(out=y.ap()[row:row + 128, :],
                                                in_=y_sb[:])

    nc.compile()
    return nc


def _emit_qkv(nc, tc, b, t):
    F32, F32R, BF16 = t["F32"], t["F32R"], t["BF16"]
    xt, wq_sb, wkv_sb = t["xt"], t["wq_sb"], t["wkv_sb"]
    cos_sb, sin_sb, id_sb = t["cos_sb"], t["sin_sb"], t["id_sb"]
    qTall, kT, v_nat = t["qTall"], t["kT"], t["v_nat"]

    with (
        tc.tile_pool(name=f"ps_acc{b}", bufs=1, space="PSUM") as ps_acc,
        tc.tile_pool(name=f"ps_T{b}", bufs=1, space="PSUM") as ps_T,
        tc.tile_pool(name=f"xtp{b}", bufs=4) as xtp,
        tc.tile_pool(name=f"rwp{b}", bufs=2) as rwp,
        tc.tile_pool(name=f"qrp{b}", bufs=2) as qrp,
        tc.tile_pool(name=f"tmp{b}", bufs=2) as tmp,
    ):
        def emit_T(prev):
            if prev is None:
                return
            g0, q_rots, k_rots = prev
            pos = g0 * 256
            for tti in range(2):
                tq = ps_T.tile([128, HDQ], F32R, tag=f"tq{tti}",
                               name=f"tq{tti}")
                for h in range(NQH):
                    nc.tensor.transpose(
                        tq[:, h * 128:(h + 1) * 128],
                        q_rots[tti][:, h * 128:(h + 1) * 128],
                        id_sb[:],
                    )
                nc.scalar.copy(
                    qTall[:].rearrange("p (h s) -> p h s", h=NQH)
                    [:, :, pos + tti * 128:pos + (tti + 1) * 128],
                    tq[:].rearrange("p (h t) -> p h t", h=NQH),
                )
            tk = ps_T.tile([128, 256], F32R, tag="tk",
                           padded_shape=[128, 512])
            for tti in range(2):
                nc.tensor.transpose(
                    tk[:, tti * 128:(tti + 1) * 128],
                    k_rots[tti][:],
                    id_sb[:],
                )
            nc.scalar.copy(kT[:, pos:pos + 256], tk[:])

        prev = None
        for g in range(NG):
            tok0 = b * S + g * 256
            psq = [ps_acc.tile([128, HDQ], F32, tag=f"psq{i}", name=f"psq{i}")
                   for i in range(2)]
            pskv = [ps_acc.tile([128, 256], F32, tag=f"pskv{i}",
                                name=f"pskv{i}", padded_shape=[128, 512])
                    for i in range(2)]
            for kcp in range(KC // 2):
                xt2 = xtp.tile([128, 512], BF16, tag="xt2")
                eng = nc.sync if kcp % 2 == 0 else nc.scalar
                eng.dma_start(
                    out=xt2[:].rearrange("p (c t) -> p c t", c=2),
                    in_=xt.ap()[kcp * 256:(kcp + 1) * 256, tok0:tok0 + 256]
                    .rearrange("(c p) t -> p c t", p=128),
                )
                for c2 in range(2):
                    kc = kcp * 2 + c2
                    for tti in range(2):
                        lhsT = xt2[:, c2 * 256 + tti * 128:
                                   c2 * 256 + (tti + 1) * 128]
                        nc.tensor.matmul(
                            psq[tti][:], lhsT,
                            wq_sb[:, kc * HDQ:(kc + 1) * HDQ],
                            start=(kc == 0), stop=(kc == KC - 1),
                        )
                        nc.tensor.matmul(
                            pskv[tti][:], lhsT,
                            wkv_sb[:, kc * 256:(kc + 1) * 256],
                            start=(kc == 0), stop=(kc == KC - 1),
                        )

            emit_T(prev)

            q_rots, k_rots = [], []
            for tti in range(2):
                gt = g * 2 + tti
                rq = rwp.tile([128, HDQ], F32, tag=f"rq{tti}", name=f"rq{tti}")
                nc.scalar.copy(rq[:], psq[tti][:])
                rkv = rwp.tile([128, 256], F32, tag=f"rkv{tti}",
                               name=f"rkv{tti}")
                nc.scalar.copy(rkv[:], pskv[tti][:])
                nc.scalar.copy(v_nat[:, gt * 128:(gt + 1) * 128],
                               rkv[:, 128:256])

                csl = slice(gt * 256, gt * 256 + 256)
                ksl = slice(gt * 256, gt * 256 + 64)
                q_rot = qrp.tile([128, HDQ], F32R, tag=f"qr{tti}",
                                 name=f"qr{tti}")
                x0 = rq[:].rearrange("p (d two) -> p d two", two=2)[:, :, 0]
                x1 = rq[:].rearrange("p (d two) -> p d two", two=2)[:, :, 1]
                r0 = q_rot[:].rearrange("p (d two) -> p d two", two=2)[:, :, 0]
                r1 = q_rot[:].rearrange("p (d two) -> p d two", two=2)[:, :, 1]
                m0 = tmp.tile([128, 256], F32, tag="m0", name="m0")
                m1 = tmp.tile([128, 256], F32, tag="m1", name="m1")
                nc.vector.tensor_mul(m0[:], x0, cos_sb[:, csl])
                nc.vector.tensor_mul(m1[:], x1, sin_sb[:, csl])
                nc.vector.tensor_sub(r0, m0[:], m1[:])
                m2 = tmp.tile([128, 256], F32, tag="m0", name="m2")
                m3 = tmp.tile([128, 256], F32, tag="m1", name="m3")
                nc.vector.tensor_mul(m2[:], x0, sin_sb[:, csl])
                nc.vector.tensor_mul(m3[:], x1, cos_sb[:, csl])
                nc.vector.tensor_add(r1, m2[:], m3[:])

                k_rot = qrp.tile([128, 128], F32R, tag=f"kr{tti}",
                                 name=f"kr{tti}")
                kx0 = rkv[:, 0:128].rearrange("p (d two) -> p d two",
                                              two=2)[:, :, 0]
                kx1 = rkv[:, 0:128].rearrange("p (d two) -> p d two",
                                              two=2)[:, :, 1]
                kr0 = k_rot[:].rearrange("p (d two) -> p d two",
                                         two=2)[:, :, 0]
                kr1 = k_rot[:].rearrange("p (d two) -> p d two",
                                         two=2)[:, :, 1]
                km0 = tmp.tile([128, 64], F32, tag="km0", name="km0")
                km1 = tmp.tile([128, 64], F32, tag="km1", name="km1")
                nc.vector.tensor_mul(km0[:], kx0, cos_sb[:, ksl])
                nc.vector.tensor_mul(km1[:], kx1, sin_sb[:, ksl])
                nc.vector.tensor_sub(kr0, km0[:], km1[:])
                km2 = tmp.tile([128, 64], F32, tag="km0", name="km2")
                km3 = tmp.tile([128, 64], F32, tag="km1", name="km3")
                nc.vector.tensor_mul(km2[:], kx0, sin_sb[:, ksl])
                nc.vector.tensor_mul(km3[:], kx1, cos_sb[:, ksl])
                nc.vector.tensor_add(kr1, km2[:], km3[:])
                q_rots.append(q_rot)
                k_rots.append(k_rot)

            prev = (g, q_rots, k_rots)
        emit_T(prev)


def _emit_attn(nc, tc, b, t):
    mybir = t["mybir"]
    F32, F32R, BF16 = t["F32"], t["F32R"], t["BF16"]
    qTall, kT, v_nat = t["qTall"], t["kT"], t["v_nat"]
    tri_sb, ones_sb, oT_h = t["tri_sb"], t["ones_sb"], t["oT_h"]

    with (
        tc.tile_pool(name=f"ps_s{b}", bufs=3, space="PSUM") as ps_s,
        tc.tile_pool(name=f"ps_o{b}", bufs=3, space="PSUM") as ps_o,
        tc.tile_pool(name=f"ps_sum{b}", bufs=2, space="PSUM") as ps_sum,
        tc.tile_pool(name=f"wa{b}", bufs=2) as wp,
        tc.tile_pool(name=f"ptp{b}", bufs=3) as ptp,
    ):
        for h in range(NQH):
            for qb in range(NQB):
                q0 = qb * QB
                kt_max = (q0 + QB) // 128 - 1
                oT = ps_o.tile([128, QB], F32, tag="oT")
                sums = ps_sum.tile([1, QB], F32, tag="sums")

                sTs = {}

                def emit_s(kt):
                    off = max(0, kt * 128 - q0)
                    qs = slice(h * S + q0 + off, h * S + q0 + QB)
                    sT = ps_s.tile([128, QB], F32, tag="sT", name="sT")
                    nc.tensor.matmul(
                        sT[:, off:QB],
                        kT[:, kt * 128:(kt + 1) * 128],
                        qTall[:, qs],
                        start=True, stop=True,
                    )
                    sTs[kt] = (sT, off)

                emit_s(0)
                for kt in range(kt_max + 1):
                    if kt < kt_max:
                        emit_s(kt + 1)
                    sT, off = sTs.pop(kt)
                    psl = slice(off, QB)
                    pT = ptp.tile([128, QB], F32R, tag="pT", name="pT")
                    nc.scalar.activation(
                        pT[:, psl], sT[:, psl],
                        mybir.ActivationFunctionType.Exp,
                        scale=SCALE,
                    )
                    if kt * 128 >= q0:
                        nc.vector.tensor_mul(
                            pT[:, off:off + 128],
                            pT[:, off:off + 128].bitcast(F32),
                            tri_sb[:],
                        )
                    nc.tensor.matmul(
                        oT[:, psl],
                        v_nat[:, kt * 128:(kt + 1) * 128],
                        pT[:, psl],
                        start=(kt == 0), stop=(kt == kt_max),
                    )
                    nc.tensor.matmul(
                        sums[0:1, psl], ones_sb[:], pT[:, psl],
                        start=(kt == 0), stop=(kt == kt_max),
                    )
                sums_sb = wp.tile([1, QB], F32, tag="sums_sb")
                nc.scalar.copy(sums_sb[:], sums[0:1, :])
                rec = wp.tile([1, QB], F32, tag="rec")
                scr = wp.tile([1, QB], F32, tag="scr")
                nc.vector.reciprocal_approx_accurate(rec[:], sums_sb[:],
                                                     scr[:])
                rb = wp.tile([128, QB], F32, tag="rb")
                nc.gpsimd.partition_broadcast(rb[:], rec[:])
                oT_sb = wp.tile([128, QB], BF16, tag="oT_sb")
                nc.vector.tensor_mul(oT_sb[:], oT[:], rb[:])
                nc.scalar.dma_start(
                    out=oT_h[b][:][h * 128:(h + 1) * 128, q0:q0 + QB],
                    in_=oT_sb[:],
                )


def _in_maps(x, wq, wk, wv, wo):
    import concourse.mybir as mybir
    np_bf16 = mybir.dt.np(mybir.dt.bfloat16)

    x2 = np.asarray(x, dtype=np.float32).reshape(T, DM)
    xT = np.ascontiguousarray(x2.T).astype(np_bf16)
    cos4, sin4, tri, ident, ones = _consts()
    wq = np.asarray(wq, np.float32)
    wk = np.asarray(wk, np.float32)
    wv = np.asarray(wv, np.float32)
    wo = np.asarray(wo, np.float32)
    maps = []
    for c in range(N_CORES):
        qsl = slice(c * HDQ, (c + 1) * HDQ)
        ksl = slice(c * HD, (c + 1) * HD)
        wkv_c = np.concatenate([wk[:, ksl], wv[:, ksl]], axis=1)
        maps.append({
            "xt": xT,
            "wq": np.ascontiguousarray(wq[:, qsl]).astype(np_bf16),
            "wkv": np.ascontiguousarray(wkv_c).astype(np_bf16),
            "wo": np.ascontiguousarray(wo[:, qsl]).astype(np_bf16),
            "cosc": cos4, "sinc": sin4, "tric": tri,
            "identc": ident, "onesc": ones,
        })
    return maps


def kernel(x, wq, wk, wv, wo, start_pos=0, **_unused):
    from concourse import bass_utils

    assert int(np.asarray(start_pos)) == 0
    in_maps = _in_maps(x, wq, wk, wv, wo)

    if "nc" not in _CACHE:
        _CACHE["nc"] = _build()
    nc = _CACHE["nc"]

    res = bass_utils.run_bass_kernel_spmd(
        nc, in_maps, core_ids=list(range(N_CORES)),
        trace=bool(int(os.environ.get("KERNEL_TRACE", "0") or 0)),
    )
    _CACHE["last_result"] = res

    out = np.empty((T, DM), np.float32)
    for c in range(N_CORES):
        out[:, c * HDQ:(c + 1) * HDQ] = res.results[c]["y"]
    return out.reshape(B, S, DM)
